# revision 5
# baseline (speedup 1.0000x reference)
"""CGGR loss (difficulty-masked CE) Trainium2 kernel, 8-way data parallel.

Self-contained: hardcodes shapes from the problem spec.
  logits (4, 2048, 32000) f32, targets (4, 2048) int, step_count scalar.
Returns the scalar f32 loss.
"""

import sys

import numpy as np

sys.path.insert(0, "/opt/trn_rl_repo")

import concourse.bacc as bacc
import concourse.bass as bass
import concourse.mybir as mybir
from concourse import bass_isa, bass_utils
from concourse.tile import TileContext

F32 = mybir.dt.float32
BF16 = mybir.dt.bfloat16
I32 = mybir.dt.int32
ALU = mybir.AluOpType
ACT = mybir.ActivationFunctionType
AX = mybir.AxisListType

MIN_TOKENS_RATIO = 0.25
WARMUP_STEPS = 1000
THRESH_SENS = 0.5

P = 128
N_CORES = 8


def _build(V, tpc, chunk, base_ratio, dve_chunks_per_row, cnt_gp_cols):
    """Build the SPMD program for one core's shard of tpc tokens."""
    rt = tpc // P            # row-tiles per core
    nch = V // chunk         # vocab chunks per row-tile
    n_tot = tpc * N_CORES    # global token count
    inv_lnv = 1.0 / float(np.log(np.float32(V)))

    nc = bacc.Bacc("TRN2", target_bir_lowering=False, debug=False,
                   num_devices=N_CORES)

    lg_d = nc.dram_tensor("logits", [tpc, V], F32, kind="ExternalInput")
    off_d = nc.dram_tensor("xt_off", [P, rt], I32, kind="ExternalInput")
    loss_d = nc.dram_tensor("loss", [1, 1], F32, kind="ExternalOutput")

    lg3 = lg_d.ap().rearrange("(p r) v -> p r v", p=P)   # token t = p*rt + r
    lg_flat = lg_d.ap().rearrange("a (b o) -> (a b) o", o=1)

    with TileContext(nc) as tc:
        with tc.tile_pool(name="stats", bufs=1) as stats, \
             tc.tile_pool(name="dram", bufs=1, space="DRAM") as dram:
            m8buf = stats.tile([P, rt, nch * 8], F32)
            zacc = stats.tile([P, rt, nch], F32)
            s1acc = stats.tile([P, rt, nch], F32)

            # ---------------- Phase 1: stream logits ----------------
            with tc.tile_pool(name="xpool", bufs=4) as xpool, \
                 tc.tile_pool(name="ppool", bufs=3) as ppool, \
                 tc.tile_pool(name="dmul", bufs=2) as dmul, \
                 tc.tile_pool(name="gmul", bufs=2) as gmul, \
                 tc.tile_pool(name="cpool", bufs=2) as cpool:
                for r in range(rt):
                    n_dve = dve_chunks_per_row[r % len(dve_chunks_per_row)]
                    for j in range(nch):
                        x = xpool.tile([P, chunk], F32, tag="x")
                        nc.sync.dma_start(
                            x[:], lg3[:, r, j * chunk:(j + 1) * chunk])
                        nc.vector.max(m8buf[:, r, j * 8:(j + 1) * 8], x[:])
                        p_t = ppool.tile([P, chunk], F32, tag="p")
                        nc.scalar.activation(p_t[:], x[:], ACT.Exp,
                                             accum_out=zacc[:, r, j:j + 1])
                        if j < n_dve:
                            xp = dmul.tile([P, chunk], F32, tag="xp")
                            nc.vector.scalar_tensor_tensor(
                                out=xp[:], in0=x[:], scalar=1.0, in1=p_t[:],
                                op0=ALU.mult, op1=ALU.mult,
                                accum_out=s1acc[:, r, j:j + 1])
                        else:
                            xpg = gmul.tile([P, chunk], F32, tag="xpg")
                            nc.gpsimd.tensor_tensor(
                                out=xpg[:], in0=x[:], in1=p_t[:], op=ALU.mult)
                            cp = cpool.tile([P, chunk], BF16, tag="cp")
                            nc.scalar.activation(
                                cp[:], xpg[:], ACT.Copy,
                                accum_out=s1acc[:, r, j:j + 1])

            # ---------------- Phase 1.5: per-token stats [P, rt] ----------
            z = stats.tile([P, rt], F32)
            s1 = stats.tile([P, rt], F32)
            nc.vector.reduce_sum(z[:], zacc[:], axis=AX.X)
            nc.vector.reduce_sum(s1[:], s1acc[:], axis=AX.X)

            top8 = stats.tile([P, rt, 8], F32)
            for r in range(rt):
                nc.vector.max(top8[:, r, :], m8buf[:, r, :])

            logz = stats.tile([P, rt], F32)
            nc.scalar.activation(logz[:], z[:], ACT.Ln)
            rz = stats.tile([P, rt], F32)
            nc.vector.reciprocal(rz[:], z[:])
            em = stats.tile([P, rt], F32)
            nc.scalar.activation(em[:], top8[:, :, 0], ACT.Exp)
            em2 = stats.tile([P, rt], F32)
            nc.scalar.activation(em2[:], top8[:, :, 1], ACT.Exp)

            conf = stats.tile([P, rt], F32)
            nc.vector.tensor_mul(conf[:], em[:], rz[:])
            margin = stats.tile([P, rt], F32)
            nc.vector.tensor_sub(margin[:], em[:], em2[:])
            nc.vector.tensor_mul(margin[:], margin[:], rz[:])

            ent = stats.tile([P, rt], F32)
            nc.vector.tensor_mul(ent[:], s1[:], rz[:])
            nc.vector.tensor_sub(ent[:], logz[:], ent[:])
            entn = stats.tile([P, rt], F32)
            nc.vector.tensor_scalar_mul(entn[:], ent[:], inv_lnv)

            # gather x[token, target] from HBM via precomputed flat offsets
            offs = stats.tile([P, rt], I32)
            nc.sync.dma_start(offs[:], off_d.ap())
            xt = stats.tile([P, rt], F32)
            for r in range(rt):
                nc.gpsimd.indirect_dma_start(
                    out=xt[:, r:r + 1], out_offset=None, in_=lg_flat,
                    in_offset=bass.IndirectOffsetOnAxis(
                        ap=offs[:, r:r + 1], axis=0))
            ce = stats.tile([P, rt], F32)
            nc.vector.tensor_sub(ce[:], logz[:], xt[:])

            # ---------------- Collective 1: avg_conf + max ce -------------
            csum = stats.tile([P, 1], F32)
            nc.vector.reduce_sum(csum[:], conf[:], axis=AX.X)
            cemx = stats.tile([P, 1], F32)
            nc.vector.reduce_max(cemx[:], ce[:], axis=AX.X)
            csum_a = stats.tile([P, 1], F32)
            nc.gpsimd.partition_all_reduce(csum_a[:], csum[:], channels=P,
                                           reduce_op=bass_isa.ReduceOp.add)
            cemx_a = stats.tile([P, 1], F32)
            nc.gpsimd.partition_all_reduce(cemx_a[:], cemx[:], channels=P,
                                           reduce_op=bass_isa.ReduceOp.max)

            sc_loc = dram.tile([1, 2], F32)
            sc_all = dram.tile([N_CORES, 2], F32, addr_space="Shared")
            nc.sync.dma_start(sc_loc[:, 0:1], csum_a[0:1, :])
            nc.sync.dma_start(sc_loc[:, 1:2], cemx_a[0:1, :])
            nc.gpsimd.collective_compute(
                "AllGather", ALU.bypass,
                replica_groups=[list(range(N_CORES))],
                ins=[sc_loc[:]], outs=[sc_all[:]])

            scg = stats.tile([P, 2 * N_CORES], F32)
            nc.sync.dma_start(
                scg[:], sc_all[:].rearrange("a b -> (a b)")
                .partition_broadcast(P))

            conf_tot = stats.tile([P, 1], F32)
            nc.vector.reduce_sum(conf_tot[:], scg[:, 0:16:2], axis=AX.X)
            cemax_g = stats.tile([P, 1], F32)
            nc.vector.reduce_max(cemax_g[:], scg[:, 1:16:2], axis=AX.X)

            # ratio -> k (round half to even like jnp.round)
            kf = stats.tile([P, 1], F32)
            nc.vector.tensor_scalar(out=kf[:], in0=conf_tot[:],
                                    scalar1=1.0 / n_tot, scalar2=-1.0,
                                    op0=ALU.mult, op1=ALU.mult)  # -avg_conf
            nc.vector.tensor_scalar(out=kf[:], in0=kf[:],
                                    scalar1=0.5, scalar2=1.0,
                                    op0=ALU.add, op1=ALU.add)  # 1+(0.5-avg)
            nc.vector.tensor_scalar_mul(kf[:], kf[:], float(base_ratio))
            nc.vector.tensor_scalar_min(kf[:], kf[:], 1.0)
            nc.vector.tensor_scalar_max(kf[:], kf[:], 0.01)
            nc.vector.tensor_scalar_mul(kf[:], kf[:], float(n_tot))
            ki = stats.tile([P, 1], I32)
            nc.vector.tensor_copy(ki[:], kf[:])      # f32->i32 RNE
            nc.vector.tensor_copy(kf[:], ki[:])      # back, exact
            nc.vector.tensor_scalar_max(kf[:], kf[:], 1.0)
            nc.vector.tensor_scalar_min(kf[:], kf[:], float(n_tot))

            # difficulty
            cmx = stats.tile([P, 1], F32)
            nc.vector.tensor_scalar_add(cmx[:], cemax_g[:], 1e-6)
            rcm = stats.tile([P, 1], F32)
            nc.vector.reciprocal(rcm[:], cmx[:])
            d_sb = stats.tile([P, rt], F32)
            nc.vector.tensor_scalar(out=d_sb[:], in0=margin[:],
                                    scalar1=-1.0, scalar2=1.0,
                                    op0=ALU.mult, op1=ALU.add)  # 1 - margin
            nc.vector.tensor_add(d_sb[:], entn[:], d_sb[:])
            lnorm = stats.tile([P, rt], F32)
            nc.vector.tensor_scalar(out=lnorm[:], in0=ce[:],
                                    scalar1=rcm[:, 0:1], scalar2=None,
                                    op0=ALU.mult)
            nc.vector.tensor_add(d_sb[:], d_sb[:], lnorm[:])
            nc.vector.tensor_scalar_mul(d_sb[:], d_sb[:], 1.0 / 3.0)

            # ---------------- Collective 2: allgather difficulties --------
            d_loc = dram.tile([1, tpc], F32)
            d_all = dram.tile([N_CORES, tpc], F32, addr_space="Shared")
            nc.sync.dma_start(
                d_loc[:].rearrange("o (p r) -> (o p) r", p=P), d_sb[:])
            nc.gpsimd.collective_compute(
                "AllGather", ALU.bypass,
                replica_groups=[list(range(N_CORES))],
                ins=[d_loc[:]], outs=[d_all[:]])

            # ---------------- Phase 2: rank counting ----------------------
            with tc.tile_pool(name="p2", bufs=1) as p2:
                d_bc = p2.tile([P, n_tot], F32)
                nc.sync.dma_start(
                    d_bc[:], d_all[:].rearrange("a b -> (a b)")
                    .partition_broadcast(P))
                cnt = stats.tile([P, rt], F32)
                for c in range(rt):
                    if c in cnt_gp_cols:
                        cg = p2.tile([P, n_tot], F32, tag="cg")
                        nc.gpsimd.tensor_scalar(
                            out=cg[:], in0=d_bc[:], scalar1=d_sb[:, c:c + 1],
                            scalar2=0.0, op0=ALU.is_gt, op1=ALU.add)
                        cb = p2.tile([P, n_tot], BF16, tag="cb")
                        nc.scalar.activation(cb[:], cg[:], ACT.Copy,
                                             accum_out=cnt[:, c:c + 1])
                    else:
                        cd = p2.tile([P, n_tot], F32, tag="cd")
                        nc.vector.tensor_scalar(
                            out=cd[:], in0=d_bc[:], scalar1=d_sb[:, c:c + 1],
                            scalar2=0.0, op0=ALU.is_gt, op1=ALU.add,
                            accum_out=cnt[:, c:c + 1])

            mask = stats.tile([P, rt], F32)
            nc.vector.tensor_scalar(out=mask[:], in0=cnt[:],
                                    scalar1=kf[:, 0:1], scalar2=None,
                                    op0=ALU.is_lt)

            tmp8 = stats.tile([P, rt], F32)
            nums = stats.tile([P, 1], F32)
            nc.vector.scalar_tensor_tensor(
                out=tmp8[:], in0=ce[:], scalar=1.0, in1=mask[:],
                op0=ALU.mult, op1=ALU.mult, accum_out=nums[:])
            dens = stats.tile([P, 1], F32)
            nc.vector.reduce_sum(dens[:], mask[:], axis=AX.X)

            num_a = stats.tile([P, 1], F32)
            nc.gpsimd.partition_all_reduce(num_a[:], nums[:], channels=P,
                                           reduce_op=bass_isa.ReduceOp.add)
            den_a = stats.tile([P, 1], F32)
            nc.gpsimd.partition_all_reduce(den_a[:], dens[:], channels=P,
                                           reduce_op=bass_isa.ReduceOp.add)

            # ---------------- Collective 3: final partials ----------------
            fin_loc = dram.tile([1, 2], F32)
            fin_all = dram.tile([N_CORES, 2], F32, addr_space="Shared")
            nc.sync.dma_start(fin_loc[:, 0:1], num_a[0:1, :])
            nc.sync.dma_start(fin_loc[:, 1:2], den_a[0:1, :])
            nc.gpsimd.collective_compute(
                "AllGather", ALU.bypass,
                replica_groups=[list(range(N_CORES))],
                ins=[fin_loc[:]], outs=[fin_all[:]])

            fing = stats.tile([P, 2 * N_CORES], F32)
            nc.sync.dma_start(
                fing[:], fin_all[:].rearrange("a b -> (a b)")
                .partition_broadcast(P))
            numt = stats.tile([P, 1], F32)
            nc.vector.reduce_sum(numt[:], fing[:, 0:16:2], axis=AX.X)
            dent = stats.tile([P, 1], F32)
            nc.vector.reduce_sum(dent[:], fing[:, 1:16:2], axis=AX.X)
            nc.vector.tensor_scalar_max(dent[:], dent[:], 1.0)
            rden = stats.tile([P, 1], F32)
            nc.vector.reciprocal(rden[:], dent[:])
            loss_t = stats.tile([P, 1], F32)
            nc.vector.tensor_mul(loss_t[:], numt[:], rden[:])
            nc.sync.dma_start(loss_d.ap(), loss_t[0:1, :])

    nc.compile()
    return nc


_CACHE = {}


def _get_program(V, tpc, chunk, base_ratio, dve_rows, cnt_gp):
    key = (V, tpc, chunk, float(base_ratio), tuple(dve_rows), tuple(cnt_gp))
    if key not in _CACHE:
        _CACHE[key] = _build(V, tpc, chunk, base_ratio, dve_rows, cnt_gp)
    return _CACHE[key]


# engine-split tuning knobs
DVE_CHUNKS_PER_ROW = (4, 3)       # of the nch vocab chunks per row-tile
CNT_GP_COLS = (4, 5, 6, 7)        # counting columns routed via gpsimd
CHUNK = 3200


def kernel(logits, targets, step_count):
    logits = np.ascontiguousarray(np.asarray(logits, dtype=np.float32))
    targets = np.asarray(targets).astype(np.int64)
    step = int(np.asarray(step_count))

    B, S, V = logits.shape
    n = B * S
    tpc = n // N_CORES
    rt = tpc // P
    lg = logits.reshape(n, V)
    tg = targets.reshape(n)

    progress = min(1.0, float(step) / WARMUP_STEPS)
    base_ratio = 1.0 - progress * (1.0 - MIN_TOKENS_RATIO)

    nc = _get_program(V, tpc, CHUNK, base_ratio, DVE_CHUNKS_PER_ROW,
                      CNT_GP_COLS)

    # per-core shards; local token t = p*rt + r lives at shard row t
    in_maps = []
    for c in range(N_CORES):
        rows = slice(c * tpc, (c + 1) * tpc)
        t_loc = np.arange(tpc, dtype=np.int64)
        off = (t_loc * V + tg.reshape(-1)[rows]).astype(np.int32)
        in_maps.append({
            "logits": lg[rows],
            "xt_off": off.reshape(P, rt),
        })

    res = bass_utils.run_bass_kernel_spmd(
        nc, in_maps, core_ids=list(range(N_CORES)), **RUN_KWARGS)
    global LAST_RESULTS
    LAST_RESULTS = res
    out = res.results[0]["loss"]
    return np.float32(out.reshape(())).astype(np.float32)


# dev hooks (unused by the grading harness)
RUN_KWARGS = {}
LAST_RESULTS = None


# revision 9
# speedup vs baseline: 1.8956x; 1.8956x over previous
"""CGGR loss (difficulty-masked CE) Trainium2 kernel, 8-way data parallel.

Self-contained: hardcodes shapes from the problem spec.
  logits (4, 2048, 32000) f32, targets (4, 2048) int, step_count scalar.
Returns the scalar f32 loss.
"""

import sys

import numpy as np

sys.path.insert(0, "/opt/trn_rl_repo")

import concourse.bacc as bacc
import concourse.bass as bass
import concourse.mybir as mybir
from concourse import bass_isa, bass_utils
from concourse.tile import TileContext

F32 = mybir.dt.float32
BF16 = mybir.dt.bfloat16
I32 = mybir.dt.int32
ALU = mybir.AluOpType
ACT = mybir.ActivationFunctionType
AX = mybir.AxisListType

MIN_TOKENS_RATIO = 0.25
WARMUP_STEPS = 1000
THRESH_SENS = 0.5

P = 128
N_CORES = 8


def _build(V, tpc, chunk, base_ratio, dve_chunks_per_row, cnt_gp_cols):
    """Build the SPMD program for one core's shard of tpc tokens."""
    rt = tpc // P            # row-tiles per core
    nch = V // chunk         # vocab chunks per row-tile
    n_tot = tpc * N_CORES    # global token count
    inv_lnv = 1.0 / float(np.log(np.float32(V)))

    nc = bacc.Bacc("TRN2", target_bir_lowering=False, debug=False,
                   num_devices=N_CORES)

    lg_d = nc.dram_tensor("logits", [tpc, V], F32, kind="ExternalInput")
    off_d = nc.dram_tensor("xt_off", [P, rt], I32, kind="ExternalInput")
    loss_d = nc.dram_tensor("loss", [1, 1], F32, kind="ExternalOutput")

    lg3 = lg_d.ap().rearrange("(p r) v -> p r v", p=P)   # token t = p*rt + r
    lg_flat = lg_d.ap().rearrange("a (b o) -> (a b) o", o=1)

    with TileContext(nc) as tc:
        with tc.tile_pool(name="stats", bufs=1) as stats, \
             tc.tile_pool(name="dram", bufs=1, space="DRAM") as dram:
            # warmup collective: absorbs first-collective fabric setup
            # (~100us) under phase 1; output is never read.
            wu_loc = dram.tile([1, 2], F32)
            wu_all = dram.tile([N_CORES, 2], F32, addr_space="Shared")
            wu_sb = stats.tile([1, 2], F32)
            nc.vector.memset(wu_sb[:], 0.0)
            nc.sync.dma_start(wu_loc[:], wu_sb[:])
            nc.gpsimd.collective_compute(
                "AllGather", ALU.bypass,
                replica_groups=[list(range(N_CORES))],
                ins=[wu_loc[:]], outs=[wu_all[:]])

            n_dve = dve_chunks_per_row[0]
            n_gp = nch - n_dve
            gp_batch = (n_gp + 1) // 2 if n_gp else 1   # chunks per batch
            n_bat = (n_gp + gp_batch - 1) // gp_batch if n_gp else 0
            m8buf = stats.tile([P, rt, nch * 8], F32)
            zacc = stats.tile([P, rt, nch], F32)
            s1acc = stats.tile([P, rt, n_dve + n_bat], F32)

            # ---------------- Phase 1: stream logits ----------------
            with tc.tile_pool(name="xpool", bufs=4) as xpool, \
                 tc.tile_pool(name="ppool", bufs=3) as ppool, \
                 tc.tile_pool(name="dmul", bufs=2) as dmul, \
                 tc.tile_pool(name="gmul", bufs=2) as gmul, \
                 tc.tile_pool(name="cpool", bufs=1) as cpool:
                pending = []      # gp product batches awaiting ACT accum
                for r in range(rt):
                    bat_tiles = {}
                    for j in range(nch):
                        x = xpool.tile([P, chunk], F32, tag="x")
                        nc.sync.dma_start(
                            x[:], lg3[:, r, j * chunk:(j + 1) * chunk])
                        nc.vector.max(m8buf[:, r, j * 8:(j + 1) * 8], x[:])
                        p_t = ppool.tile([P, chunk], F32, tag="p")
                        nc.scalar.activation(p_t[:], x[:], ACT.Exp,
                                             accum_out=zacc[:, r, j:j + 1])
                        if j < n_dve:
                            xp = dmul.tile([P, chunk], F32, tag="xp")
                            nc.vector.scalar_tensor_tensor(
                                out=xp[:], in0=x[:], scalar=1.0, in1=p_t[:],
                                op0=ALU.mult, op1=ALU.mult,
                                accum_out=s1acc[:, r, j:j + 1])
                        else:
                            g = j - n_dve
                            b, slot = divmod(g, gp_batch)
                            bw = min(gp_batch, n_gp - b * gp_batch)
                            if slot == 0:
                                bat_tiles[b] = gmul.tile(
                                    [P, gp_batch * chunk], BF16, tag="gpb",
                                    name=f"gpb_{r}_{b}")
                            xpg = bat_tiles[b]
                            nc.gpsimd.tensor_tensor(
                                out=xpg[:, slot * chunk:(slot + 1) * chunk],
                                in0=x[:], in1=p_t[:], op=ALU.mult)
                            if slot == bw - 1:
                                pending.append((xpg, bw * chunk, r, n_dve + b))
                    # flush previous row's gp batches (lagged: ACT is past
                    # the gp mults by now, so no ACT<->GPSIMD ping-pong)
                    while len(pending) > (n_bat if r < rt - 1 else 0):
                        xpg, w, rr, col = pending.pop(0)
                        cp = cpool.tile([P, gp_batch * chunk], BF16, tag="cp")
                        nc.scalar.activation(
                            cp[:, :w], xpg[:, :w], ACT.Copy,
                            accum_out=s1acc[:, rr, col:col + 1])

            # ---------------- Phase 1.5: per-token stats [P, rt] ----------
            z = stats.tile([P, rt], F32)
            s1 = stats.tile([P, rt], F32)
            nc.vector.reduce_sum(z[:], zacc[:], axis=AX.X)
            nc.vector.reduce_sum(s1[:], s1acc[:], axis=AX.X)

            top8 = stats.tile([P, rt, 8], F32)
            for r in range(rt):
                nc.vector.max(top8[:, r, :], m8buf[:, r, :])

            logz = stats.tile([P, rt], F32)
            nc.scalar.activation(logz[:], z[:], ACT.Ln)
            rz = stats.tile([P, rt], F32)
            nc.vector.reciprocal(rz[:], z[:])
            em = stats.tile([P, rt], F32)
            nc.scalar.activation(em[:], top8[:, :, 0], ACT.Exp)
            em2 = stats.tile([P, rt], F32)
            nc.scalar.activation(em2[:], top8[:, :, 1], ACT.Exp)

            conf = stats.tile([P, rt], F32)
            nc.vector.tensor_mul(conf[:], em[:], rz[:])
            margin = stats.tile([P, rt], F32)
            nc.vector.tensor_sub(margin[:], em[:], em2[:])
            nc.vector.tensor_mul(margin[:], margin[:], rz[:])

            ent = stats.tile([P, rt], F32)
            nc.vector.tensor_mul(ent[:], s1[:], rz[:])
            nc.vector.tensor_sub(ent[:], logz[:], ent[:])
            entn = stats.tile([P, rt], F32)
            nc.vector.tensor_scalar_mul(entn[:], ent[:], inv_lnv)

            # gather x[token, target] from HBM via precomputed flat offsets
            offs = stats.tile([P, rt], I32)
            nc.sync.dma_start(offs[:], off_d.ap())
            xt = stats.tile([P, rt], F32)
            for r in range(rt):
                nc.gpsimd.indirect_dma_start(
                    out=xt[:, r:r + 1], out_offset=None, in_=lg_flat,
                    in_offset=bass.IndirectOffsetOnAxis(
                        ap=offs[:, r:r + 1], axis=0))
            ce = stats.tile([P, rt], F32)
            nc.vector.tensor_sub(ce[:], logz[:], xt[:])

            # ---------------- Collective 1: avg_conf + max ce -------------
            csum = stats.tile([P, 1], F32)
            nc.vector.reduce_sum(csum[:], conf[:], axis=AX.X)
            cemx = stats.tile([P, 1], F32)
            nc.vector.reduce_max(cemx[:], ce[:], axis=AX.X)
            csum_a = stats.tile([P, 1], F32)
            nc.gpsimd.partition_all_reduce(csum_a[:], csum[:], channels=P,
                                           reduce_op=bass_isa.ReduceOp.add)
            cemx_a = stats.tile([P, 1], F32)
            nc.gpsimd.partition_all_reduce(cemx_a[:], cemx[:], channels=P,
                                           reduce_op=bass_isa.ReduceOp.max)

            sc_loc = dram.tile([1, 2], F32)
            sc_all = dram.tile([N_CORES, 2], F32, addr_space="Shared")
            nc.sync.dma_start(sc_loc[:, 0:1], csum_a[0:1, :])
            nc.sync.dma_start(sc_loc[:, 1:2], cemx_a[0:1, :])
            nc.gpsimd.collective_compute(
                "AllGather", ALU.bypass,
                replica_groups=[list(range(N_CORES))],
                ins=[sc_loc[:]], outs=[sc_all[:]])

            scg = stats.tile([P, 2 * N_CORES], F32)
            nc.sync.dma_start(
                scg[:], sc_all[:].rearrange("a b -> (a b)")
                .partition_broadcast(P))

            conf_tot = stats.tile([P, 1], F32)
            nc.vector.reduce_sum(conf_tot[:], scg[:, 0:16:2], axis=AX.X)
            cemax_g = stats.tile([P, 1], F32)
            nc.vector.reduce_max(cemax_g[:], scg[:, 1:16:2], axis=AX.X)

            # ratio -> k (round half to even like jnp.round)
            kf = stats.tile([P, 1], F32)
            nc.vector.tensor_scalar(out=kf[:], in0=conf_tot[:],
                                    scalar1=1.0 / n_tot, scalar2=-1.0,
                                    op0=ALU.mult, op1=ALU.mult)  # -avg_conf
            nc.vector.tensor_scalar(out=kf[:], in0=kf[:],
                                    scalar1=0.5, scalar2=1.0,
                                    op0=ALU.add, op1=ALU.add)  # 1+(0.5-avg)
            nc.vector.tensor_scalar_mul(kf[:], kf[:], float(base_ratio))
            nc.vector.tensor_scalar_min(kf[:], kf[:], 1.0)
            nc.vector.tensor_scalar_max(kf[:], kf[:], 0.01)
            nc.vector.tensor_scalar_mul(kf[:], kf[:], float(n_tot))
            ki = stats.tile([P, 1], I32)
            nc.vector.tensor_copy(ki[:], kf[:])      # f32->i32 RNE
            nc.vector.tensor_copy(kf[:], ki[:])      # back, exact
            nc.vector.tensor_scalar_max(kf[:], kf[:], 1.0)
            nc.vector.tensor_scalar_min(kf[:], kf[:], float(n_tot))

            # difficulty
            cmx = stats.tile([P, 1], F32)
            nc.vector.tensor_scalar_add(cmx[:], cemax_g[:], 1e-6)
            rcm = stats.tile([P, 1], F32)
            nc.vector.reciprocal(rcm[:], cmx[:])
            d_sb = stats.tile([P, rt], F32)
            nc.vector.tensor_scalar(out=d_sb[:], in0=margin[:],
                                    scalar1=-1.0, scalar2=1.0,
                                    op0=ALU.mult, op1=ALU.add)  # 1 - margin
            nc.vector.tensor_add(d_sb[:], entn[:], d_sb[:])
            lnorm = stats.tile([P, rt], F32)
            nc.vector.tensor_scalar(out=lnorm[:], in0=ce[:],
                                    scalar1=rcm[:, 0:1], scalar2=None,
                                    op0=ALU.mult)
            nc.vector.tensor_add(d_sb[:], d_sb[:], lnorm[:])
            nc.vector.tensor_scalar_mul(d_sb[:], d_sb[:], 1.0 / 3.0)

            # ---------------- Collective 2: allgather difficulties --------
            d_loc = dram.tile([1, tpc], F32)
            d_all = dram.tile([N_CORES, tpc], F32, addr_space="Shared")
            nc.sync.dma_start(
                d_loc[:].rearrange("o (p r) -> (o p) r", p=P), d_sb[:])
            nc.gpsimd.collective_compute(
                "AllGather", ALU.bypass,
                replica_groups=[list(range(N_CORES))],
                ins=[d_loc[:]], outs=[d_all[:]])

            # ---------------- Phase 2: rank counting ----------------------
            with tc.tile_pool(name="p2", bufs=1) as p2:
                d_bc = p2.tile([P, n_tot], F32)
                nc.sync.dma_start(
                    d_bc[:], d_all[:].rearrange("a b -> (a b)")
                    .partition_broadcast(P))
                cnt = stats.tile([P, rt], F32)
                for c in range(rt):
                    if c in cnt_gp_cols:
                        cg = p2.tile([P, n_tot], F32, tag="cg")
                        nc.gpsimd.tensor_scalar(
                            out=cg[:], in0=d_bc[:], scalar1=d_sb[:, c:c + 1],
                            scalar2=0.0, op0=ALU.is_gt, op1=ALU.add)
                        cb = p2.tile([P, n_tot], BF16, tag="cb")
                        nc.scalar.activation(cb[:], cg[:], ACT.Copy,
                                             accum_out=cnt[:, c:c + 1])
                    else:
                        cd = p2.tile([P, n_tot], F32, tag="cd")
                        nc.vector.tensor_scalar(
                            out=cd[:], in0=d_bc[:], scalar1=d_sb[:, c:c + 1],
                            scalar2=0.0, op0=ALU.is_gt, op1=ALU.add,
                            accum_out=cnt[:, c:c + 1])

            mask = stats.tile([P, rt], F32)
            nc.vector.tensor_scalar(out=mask[:], in0=cnt[:],
                                    scalar1=kf[:, 0:1], scalar2=None,
                                    op0=ALU.is_lt)

            tmp8 = stats.tile([P, rt], F32)
            nums = stats.tile([P, 1], F32)
            nc.vector.scalar_tensor_tensor(
                out=tmp8[:], in0=ce[:], scalar=1.0, in1=mask[:],
                op0=ALU.mult, op1=ALU.mult, accum_out=nums[:])
            dens = stats.tile([P, 1], F32)
            nc.vector.reduce_sum(dens[:], mask[:], axis=AX.X)

            num_a = stats.tile([P, 1], F32)
            nc.gpsimd.partition_all_reduce(num_a[:], nums[:], channels=P,
                                           reduce_op=bass_isa.ReduceOp.add)
            den_a = stats.tile([P, 1], F32)
            nc.gpsimd.partition_all_reduce(den_a[:], dens[:], channels=P,
                                           reduce_op=bass_isa.ReduceOp.add)

            # ---------------- Collective 3: final partials ----------------
            fin_loc = dram.tile([1, 2], F32)
            fin_all = dram.tile([N_CORES, 2], F32, addr_space="Shared")
            nc.sync.dma_start(fin_loc[:, 0:1], num_a[0:1, :])
            nc.sync.dma_start(fin_loc[:, 1:2], den_a[0:1, :])
            nc.gpsimd.collective_compute(
                "AllGather", ALU.bypass,
                replica_groups=[list(range(N_CORES))],
                ins=[fin_loc[:]], outs=[fin_all[:]])

            fing = stats.tile([P, 2 * N_CORES], F32)
            nc.sync.dma_start(
                fing[:], fin_all[:].rearrange("a b -> (a b)")
                .partition_broadcast(P))
            numt = stats.tile([P, 1], F32)
            nc.vector.reduce_sum(numt[:], fing[:, 0:16:2], axis=AX.X)
            dent = stats.tile([P, 1], F32)
            nc.vector.reduce_sum(dent[:], fing[:, 1:16:2], axis=AX.X)
            nc.vector.tensor_scalar_max(dent[:], dent[:], 1.0)
            rden = stats.tile([P, 1], F32)
            nc.vector.reciprocal(rden[:], dent[:])
            loss_t = stats.tile([P, 1], F32)
            nc.vector.tensor_mul(loss_t[:], numt[:], rden[:])
            nc.sync.dma_start(loss_d.ap(), loss_t[0:1, :])

    nc.compile()
    return nc


_CACHE = {}


def _get_program(V, tpc, chunk, base_ratio, dve_rows, cnt_gp):
    key = (V, tpc, chunk, float(base_ratio), tuple(dve_rows), tuple(cnt_gp))
    if key not in _CACHE:
        _CACHE[key] = _build(V, tpc, chunk, base_ratio, dve_rows, cnt_gp)
    return _CACHE[key]


# engine-split tuning knobs
DVE_CHUNKS_PER_ROW = (4,)         # of the nch vocab chunks per row-tile
CNT_GP_COLS = ()                  # counting columns routed via gpsimd
CHUNK = 3200


def kernel(logits, targets, step_count):
    logits = np.ascontiguousarray(np.asarray(logits, dtype=np.float32))
    targets = np.asarray(targets).astype(np.int64)
    step = int(np.asarray(step_count))

    B, S, V = logits.shape
    n = B * S
    tpc = n // N_CORES
    rt = tpc // P
    lg = logits.reshape(n, V)
    tg = targets.reshape(n)

    progress = min(1.0, float(step) / WARMUP_STEPS)
    base_ratio = 1.0 - progress * (1.0 - MIN_TOKENS_RATIO)

    nc = _get_program(V, tpc, CHUNK, base_ratio, DVE_CHUNKS_PER_ROW,
                      CNT_GP_COLS)

    # per-core shards; local token t = p*rt + r lives at shard row t
    in_maps = []
    for c in range(N_CORES):
        rows = slice(c * tpc, (c + 1) * tpc)
        t_loc = np.arange(tpc, dtype=np.int64)
        off = (t_loc * V + tg.reshape(-1)[rows]).astype(np.int32)
        in_maps.append({
            "logits": lg[rows],
            "xt_off": off.reshape(P, rt),
        })

    res = bass_utils.run_bass_kernel_spmd(
        nc, in_maps, core_ids=list(range(N_CORES)), **RUN_KWARGS)
    global LAST_RESULTS
    LAST_RESULTS = res
    out = res.results[0]["loss"]
    return np.float32(out.reshape(())).astype(np.float32)


# dev hooks (unused by the grading harness)
RUN_KWARGS = {}
LAST_RESULTS = None


# revision 17
# speedup vs baseline: 2.0071x; 1.0588x over previous
"""CGGR loss (difficulty-masked CE) Trainium2 kernel, 8-way data parallel.

Self-contained: hardcodes shapes from the problem spec.
  logits (4, 2048, 32000) f32, targets (4, 2048) int, step_count scalar.
Returns the scalar f32 loss.
"""

import sys

import numpy as np

sys.path.insert(0, "/opt/trn_rl_repo")

import concourse.bacc as bacc
import concourse.bass as bass
import concourse.mybir as mybir
from concourse import bass_isa, bass_utils
from concourse.tile import TileContext

F32 = mybir.dt.float32
BF16 = mybir.dt.bfloat16
I32 = mybir.dt.int32
ALU = mybir.AluOpType
ACT = mybir.ActivationFunctionType
AX = mybir.AxisListType

MIN_TOKENS_RATIO = 0.25
WARMUP_STEPS = 1000
THRESH_SENS = 0.5

P = 128
N_CORES = 8


def _build(V, tpc, chunk, base_ratio, dve_chunks_per_row, cnt_gp_cols):
    """Build the SPMD program for one core's shard of tpc tokens."""
    rt = tpc // P            # row-tiles per core
    nch = V // chunk         # vocab chunks per row-tile
    n_tot = tpc * N_CORES    # global token count
    inv_lnv = 1.0 / float(np.log(np.float32(V)))

    nc = bacc.Bacc("TRN2", target_bir_lowering=False, debug=False,
                   num_devices=N_CORES)

    lg_d = nc.dram_tensor("logits", [tpc, V], F32, kind="ExternalInput")
    off_d = nc.dram_tensor("xt_off", [P, rt], I32, kind="ExternalInput")
    loss_d = nc.dram_tensor("loss", [1, 1], F32, kind="ExternalOutput")
    dbg_d = nc.dram_tensor("dbg", [1, 24], F32, kind="ExternalOutput")

    lg3 = lg_d.ap().rearrange("(p r) v -> p r v", p=P)   # token t = p*rt + r
    lg_flat = lg_d.ap().rearrange("a (b o) -> (a b) o", o=1)

    with TileContext(nc) as tc:
        with tc.tile_pool(name="stats", bufs=1) as stats, \
             tc.tile_pool(name="dram", bufs=1, space="DRAM") as dram:
            # warmup collective: absorbs first-collective fabric setup
            # (~100us) under phase 1; output is never read.
            wu_loc = dram.tile([1, 2], F32)
            wu_all = dram.tile([N_CORES, 2], F32, addr_space="Shared")
            wu_sb = stats.tile([1, 2], F32)
            nc.vector.memset(wu_sb[:], 0.0)
            nc.sync.dma_start(wu_loc[:], wu_sb[:])
            nc.gpsimd.collective_compute(
                "AllGather", ALU.bypass,
                replica_groups=[list(range(N_CORES))],
                ins=[wu_loc[:]], outs=[wu_all[:]])

            n_dve = dve_chunks_per_row[0]
            n_gp = nch - n_dve
            gp_batch = (n_gp + 1) // 2 if n_gp else 1   # chunks per batch
            n_bat = (n_gp + gp_batch - 1) // gp_batch if n_gp else 0
            m8buf = stats.tile([P, rt, nch * 8], F32)
            zacc = stats.tile([P, rt, nch], F32)
            s1acc = stats.tile([P, rt, n_dve + n_bat], F32)

            # ---------------- Phase 1: stream logits ----------------
            with tc.tile_pool(name="xpool", bufs=5) as xpool, \
                 tc.tile_pool(name="ppool", bufs=6) as ppool, \
                 tc.tile_pool(name="dmul", bufs=2) as dmul, \
                 tc.tile_pool(name="gmul", bufs=2) as gmul, \
                 tc.tile_pool(name="cpool", bufs=1) as cpool:
                pending = []      # gp product batches awaiting ACT accum
                for r in range(rt):
                    bat_tiles = {}
                    for j in range(nch):
                        x = xpool.tile([P, chunk], F32, tag="x")
                        nc.sync.dma_start(
                            x[:], lg3[:, r, j * chunk:(j + 1) * chunk])
                        nc.vector.max(m8buf[:, r, j * 8:(j + 1) * 8], x[:])
                        p_t = ppool.tile([P, chunk], BF16, tag="p")
                        nc.scalar.activation(p_t[:], x[:], ACT.Exp,
                                             accum_out=zacc[:, r, j:j + 1])
                        if j < n_dve:
                            xp = dmul.tile([P, chunk], F32, tag="xp")
                            nc.vector.scalar_tensor_tensor(
                                out=xp[:], in0=x[:], scalar=1.0, in1=p_t[:],
                                op0=ALU.mult, op1=ALU.mult,
                                accum_out=s1acc[:, r, j:j + 1])
                        else:
                            g = j - n_dve
                            b, slot = divmod(g, gp_batch)
                            bw = min(gp_batch, n_gp - b * gp_batch)
                            if slot == 0:
                                bat_tiles[b] = gmul.tile(
                                    [P, gp_batch * chunk], BF16, tag="gpb",
                                    name=f"gpb_{r}_{b}")
                            xpg = bat_tiles[b]
                            nc.gpsimd.tensor_tensor(
                                out=xpg[:, slot * chunk:(slot + 1) * chunk],
                                in0=x[:], in1=p_t[:], op=ALU.mult)
                            if slot == bw - 1:
                                pending.append((xpg, bw * chunk, r, n_dve + b))
                    # flush previous row's gp batches (lagged: ACT is past
                    # the gp mults by now, so no ACT<->GPSIMD ping-pong)
                    while len(pending) > (n_bat if r < rt - 1 else 0):
                        xpg, w, rr, col = pending.pop(0)
                        cp = cpool.tile([P, gp_batch * chunk], BF16, tag="cp")
                        nc.scalar.activation(
                            cp[:, :w], xpg[:, :w], ACT.Copy,
                            accum_out=s1acc[:, rr, col:col + 1])

            # ---------------- Phase 1.5: per-token stats [P, rt] ----------
            z = stats.tile([P, rt], F32)
            s1 = stats.tile([P, rt], F32)
            nc.vector.reduce_sum(z[:], zacc[:], axis=AX.X)
            nc.vector.reduce_sum(s1[:], s1acc[:], axis=AX.X)

            top8 = stats.tile([P, rt, 8], F32)
            for r in range(rt):
                nc.vector.max(top8[:, r, :], m8buf[:, r, :])

            logz = stats.tile([P, rt], F32)
            nc.scalar.activation(logz[:], z[:], ACT.Ln)
            rz = stats.tile([P, rt], F32)
            nc.vector.reciprocal(rz[:], z[:])
            em = stats.tile([P, rt], F32)
            nc.scalar.activation(em[:], top8[:, :, 0], ACT.Exp)
            em2 = stats.tile([P, rt], F32)
            nc.scalar.activation(em2[:], top8[:, :, 1], ACT.Exp)

            conf = stats.tile([P, rt], F32)
            nc.vector.tensor_mul(conf[:], em[:], rz[:])
            margin = stats.tile([P, rt], F32)
            nc.vector.tensor_sub(margin[:], em[:], em2[:])
            nc.vector.tensor_mul(margin[:], margin[:], rz[:])

            ent = stats.tile([P, rt], F32)
            nc.vector.tensor_mul(ent[:], s1[:], rz[:])
            nc.vector.tensor_sub(ent[:], logz[:], ent[:])
            entn = stats.tile([P, rt], F32)
            nc.vector.tensor_scalar_mul(entn[:], ent[:], inv_lnv)

            # gather x[token, target] from HBM via precomputed flat offsets
            offs = stats.tile([P, rt], I32)
            nc.sync.dma_start(offs[:], off_d.ap())
            xt = stats.tile([P, rt], F32)
            for r in range(rt):
                nc.gpsimd.indirect_dma_start(
                    out=xt[:, r:r + 1], out_offset=None, in_=lg_flat,
                    in_offset=bass.IndirectOffsetOnAxis(
                        ap=offs[:, r:r + 1], axis=0))
            ce = stats.tile([P, rt], F32)
            nc.vector.tensor_sub(ce[:], logz[:], xt[:])

            # ---------------- Collective 1: avg_conf + max ce -------------
            csum = stats.tile([P, 1], F32)
            nc.vector.reduce_sum(csum[:], conf[:], axis=AX.X)
            cemx = stats.tile([P, 1], F32)
            nc.vector.reduce_max(cemx[:], ce[:], axis=AX.X)
            csum_a = stats.tile([P, 1], F32)
            nc.gpsimd.partition_all_reduce(csum_a[:], csum[:], channels=P,
                                           reduce_op=bass_isa.ReduceOp.add)
            cemx_a = stats.tile([P, 1], F32)
            nc.gpsimd.partition_all_reduce(cemx_a[:], cemx[:], channels=P,
                                           reduce_op=bass_isa.ReduceOp.max)

            wu_rd = stats.tile([1, 2], F32)
            nc.sync.dma_start(wu_rd[:], wu_all[0:1, :])
            nc.vector.tensor_scalar(out=csum_a[0:1, :], in0=csum_a[0:1, :],
                                    scalar1=wu_rd[0:1, 0:1], scalar2=None,
                                    op0=ALU.add)
            nc.vector.tensor_scalar(out=cemx_a[0:1, :], in0=cemx_a[0:1, :],
                                    scalar1=wu_rd[0:1, 1:2], scalar2=None,
                                    op0=ALU.add)
            sc_loc = dram.tile([1, 2], F32)
            sc_all = dram.tile([N_CORES, 2], F32, addr_space="Shared")
            nc.sync.dma_start(sc_loc[:, 0:1], csum_a[0:1, :])
            nc.sync.dma_start(sc_loc[:, 1:2], cemx_a[0:1, :])
            nc.gpsimd.collective_compute(
                "AllGather", ALU.bypass,
                replica_groups=[list(range(N_CORES))],
                ins=[sc_loc[:]], outs=[sc_all[:]])

            scg = stats.tile([P, 2 * N_CORES], F32)
            nc.sync.dma_start(
                scg[:], sc_all[:].rearrange("a b -> (a b)")
                .partition_broadcast(P))

            conf_tot = stats.tile([P, 1], F32)
            nc.vector.reduce_sum(conf_tot[:], scg[:, 0:16:2], axis=AX.X)
            cemax_g = stats.tile([P, 1], F32)
            nc.vector.reduce_max(cemax_g[:], scg[:, 1:16:2], axis=AX.X)

            # ratio -> k (round half to even like jnp.round)
            kf = stats.tile([P, 1], F32)
            nc.vector.tensor_scalar(out=kf[:], in0=conf_tot[:],
                                    scalar1=1.0 / n_tot, scalar2=-1.0,
                                    op0=ALU.mult, op1=ALU.mult)  # -avg_conf
            nc.vector.tensor_scalar(out=kf[:], in0=kf[:],
                                    scalar1=0.5, scalar2=1.0,
                                    op0=ALU.add, op1=ALU.add)  # 1+(0.5-avg)
            nc.vector.tensor_scalar_mul(kf[:], kf[:], float(base_ratio))
            nc.vector.tensor_scalar_min(kf[:], kf[:], 1.0)
            nc.vector.tensor_scalar_max(kf[:], kf[:], 0.01)
            nc.vector.tensor_scalar_mul(kf[:], kf[:], float(n_tot))
            ki = stats.tile([P, 1], I32)
            nc.vector.tensor_copy(ki[:], kf[:])      # f32->i32 RNE
            nc.vector.tensor_copy(kf[:], ki[:])      # back, exact
            nc.vector.tensor_scalar_max(kf[:], kf[:], 1.0)
            nc.vector.tensor_scalar_min(kf[:], kf[:], float(n_tot))

            # difficulty
            cmx = stats.tile([P, 1], F32)
            nc.vector.tensor_scalar_add(cmx[:], cemax_g[:], 1e-6)
            rcm = stats.tile([P, 1], F32)
            nc.vector.reciprocal(rcm[:], cmx[:])
            d_sb = stats.tile([P, rt], F32)
            nc.vector.tensor_scalar(out=d_sb[:], in0=margin[:],
                                    scalar1=-1.0, scalar2=1.0,
                                    op0=ALU.mult, op1=ALU.add)  # 1 - margin
            nc.vector.tensor_add(d_sb[:], entn[:], d_sb[:])
            lnorm = stats.tile([P, rt], F32)
            nc.vector.tensor_scalar(out=lnorm[:], in0=ce[:],
                                    scalar1=rcm[:, 0:1], scalar2=None,
                                    op0=ALU.mult)
            nc.vector.tensor_add(d_sb[:], d_sb[:], lnorm[:])
            nc.vector.tensor_scalar_mul(d_sb[:], d_sb[:], 1.0 / 3.0)

            # ---------------- Collective 2: allgather difficulties --------
            d_loc = dram.tile([1, tpc], F32)
            d_all = dram.tile([N_CORES, tpc], F32, addr_space="Shared")
            nc.sync.dma_start(
                d_loc[:].rearrange("o (p r) -> (o p) r", p=P), d_sb[:])
            nc.gpsimd.collective_compute(
                "AllGather", ALU.bypass,
                replica_groups=[list(range(N_CORES))],
                ins=[d_loc[:]], outs=[d_all[:]])

            # ---------------- Phase 2: rank counting ----------------------
            dck = stats.tile([P, 1], F32)
            with tc.tile_pool(name="p2", bufs=1) as p2:
                d_bc = p2.tile([P, n_tot], F32)
                nc.sync.dma_start(
                    d_bc[:], d_all[:].rearrange("a b -> (a b)")
                    .partition_broadcast(P))
                nc.vector.reduce_sum(dck[:], d_bc[:], axis=AX.X)
                cnt = stats.tile([P, rt], F32)
                for c in range(rt):
                    if c in cnt_gp_cols:
                        cg = p2.tile([P, n_tot], BF16, tag="cg")
                        nc.gpsimd.tensor_tensor(
                            out=cg[:], in0=d_bc[:],
                            in1=d_sb[:, c:c + 1].to_broadcast([P, n_tot]),
                            op=ALU.is_gt)
                        cb = p2.tile([P, n_tot], BF16, tag="cb")
                        nc.scalar.activation(cb[:], cg[:], ACT.Copy,
                                             accum_out=cnt[:, c:c + 1])
                    else:
                        cd = p2.tile([P, n_tot], F32, tag="cd")
                        nc.vector.tensor_scalar(
                            out=cd[:], in0=d_bc[:], scalar1=d_sb[:, c:c + 1],
                            scalar2=0.0, op0=ALU.is_gt, op1=ALU.add,
                            accum_out=cnt[:, c:c + 1])

            mask = stats.tile([P, rt], F32)
            nc.vector.tensor_scalar(out=mask[:], in0=cnt[:],
                                    scalar1=kf[:, 0:1], scalar2=None,
                                    op0=ALU.is_lt)

            tmp8 = stats.tile([P, rt], F32)
            nums = stats.tile([P, 1], F32)
            nc.vector.scalar_tensor_tensor(
                out=tmp8[:], in0=ce[:], scalar=1.0, in1=mask[:],
                op0=ALU.mult, op1=ALU.mult, accum_out=nums[:])
            dens = stats.tile([P, 1], F32)
            nc.vector.reduce_sum(dens[:], mask[:], axis=AX.X)

            num_a = stats.tile([P, 1], F32)
            nc.gpsimd.partition_all_reduce(num_a[:], nums[:], channels=P,
                                           reduce_op=bass_isa.ReduceOp.add)
            den_a = stats.tile([P, 1], F32)
            nc.gpsimd.partition_all_reduce(den_a[:], dens[:], channels=P,
                                           reduce_op=bass_isa.ReduceOp.add)

            # ---------------- Collective 3: final partials ----------------
            fin_loc = dram.tile([1, 2], F32)
            fin_all = dram.tile([N_CORES, 2], F32, addr_space="Shared")
            nc.sync.dma_start(fin_loc[:, 0:1], num_a[0:1, :])
            nc.sync.dma_start(fin_loc[:, 1:2], den_a[0:1, :])
            nc.gpsimd.collective_compute(
                "AllGather", ALU.bypass,
                replica_groups=[list(range(N_CORES))],
                ins=[fin_loc[:]], outs=[fin_all[:]])

            fing = stats.tile([P, 2 * N_CORES], F32)
            nc.sync.dma_start(
                fing[:], fin_all[:].rearrange("a b -> (a b)")
                .partition_broadcast(P))
            numt = stats.tile([P, 1], F32)
            nc.vector.reduce_sum(numt[:], fing[:, 0:16:2], axis=AX.X)
            dent = stats.tile([P, 1], F32)
            nc.vector.reduce_sum(dent[:], fing[:, 1:16:2], axis=AX.X)
            nc.vector.tensor_scalar_max(dent[:], dent[:], 1.0)
            rden = stats.tile([P, 1], F32)
            nc.vector.reciprocal(rden[:], dent[:])
            loss_t = stats.tile([P, 1], F32)
            nc.vector.tensor_mul(loss_t[:], numt[:], rden[:])
            nc.sync.dma_start(loss_d.ap(), loss_t[0:1, :])

            # debug block: collective-1 view, k, d/count checksums
            nc.sync.dma_start(dbg_d.ap()[:, 0:16], scg[0:1, :])
            nc.sync.dma_start(dbg_d.ap()[:, 16:17], kf[0:1, :])
            nc.sync.dma_start(dbg_d.ap()[:, 17:18], dck[0:1, :])
            cck = stats.tile([P, 1], F32)
            nc.vector.reduce_sum(cck[:], cnt[:], axis=AX.X)
            nc.sync.dma_start(dbg_d.ap()[:, 18:19], cck[0:1, :])
            nc.sync.dma_start(dbg_d.ap()[:, 19:20], nums[0:1, :])
            nc.sync.dma_start(dbg_d.ap()[:, 20:21], dens[0:1, :])
            nc.sync.dma_start(dbg_d.ap()[:, 21:22], num_a[0:1, :])
            nc.sync.dma_start(dbg_d.ap()[:, 22:23], den_a[0:1, :])
            zck = stats.tile([P, 1], F32)
            nc.vector.reduce_sum(zck[:], s1[:], axis=AX.X)
            nc.sync.dma_start(dbg_d.ap()[:, 23:24], zck[0:1, :])

    nc.compile()
    return nc


_CACHE = {}


def _get_program(V, tpc, chunk, base_ratio, dve_rows, cnt_gp):
    key = (V, tpc, chunk, float(base_ratio), tuple(dve_rows), tuple(cnt_gp))
    if key not in _CACHE:
        _CACHE[key] = _build(V, tpc, chunk, base_ratio, dve_rows, cnt_gp)
    return _CACHE[key]


# engine-split tuning knobs
DVE_CHUNKS_PER_ROW = (3,)         # of the nch vocab chunks per row-tile
CNT_GP_COLS = ()                  # counting columns routed via gpsimd
CHUNK = 3200


def kernel(logits, targets, step_count):
    logits = np.ascontiguousarray(np.asarray(logits, dtype=np.float32))
    targets = np.asarray(targets).astype(np.int64)
    step = int(np.asarray(step_count))

    B, S, V = logits.shape
    n = B * S
    tpc = n // N_CORES
    rt = tpc // P
    lg = logits.reshape(n, V)
    tg = targets.reshape(n)

    progress = min(1.0, float(step) / WARMUP_STEPS)
    base_ratio = 1.0 - progress * (1.0 - MIN_TOKENS_RATIO)

    nc = _get_program(V, tpc, CHUNK, base_ratio, DVE_CHUNKS_PER_ROW,
                      CNT_GP_COLS)

    # per-core shards; local token t = p*rt + r lives at shard row t
    in_maps = []
    for c in range(N_CORES):
        rows = slice(c * tpc, (c + 1) * tpc)
        t_loc = np.arange(tpc, dtype=np.int64)
        off = (t_loc * V + tg.reshape(-1)[rows]).astype(np.int32)
        in_maps.append({
            "logits": lg[rows],
            "xt_off": off.reshape(P, rt),
        })

    res = bass_utils.run_bass_kernel_spmd(
        nc, in_maps, core_ids=list(range(N_CORES)), **RUN_KWARGS)
    global LAST_RESULTS
    LAST_RESULTS = res
    out = res.results[0]["loss"]
    return np.float32(out.reshape(())).astype(np.float32)


# dev hooks (unused by the grading harness)
RUN_KWARGS = {}
LAST_RESULTS = None
